# revision 78
# baseline (speedup 1.0000x reference)
"""Trainium2 Bass kernel for nn_LongTermMemory (retrieval_knn).

reference: best[b] = argmax_m cos(context[b], memory[m]); return
memory[best][None] -> [1, B, D].

Strategy (8 NeuronCores, memory sharded on M -> 8192 rows/core):
  Host prep (cheap numpy, all inside kernel()):
    - L2-normalize memory rows and context rows in fp32, scale by 64,
      quantize to fp8 e4m3, transpose to d-major layout per core.
  Device per core (screening only, fp8 in / f32 PSUM / fp16 out):
    - fp8 DoubleRow matmuls: sim[b 128, m 512] f32 in PSUM, K=512 in
      2 instructions (256 contraction each); 2 PSUM banks per b-chunk.
    - Drain + max-fold the 16 m-groups of each b-chunk into a 2-lane
      [128, 2, 512] running-max array (lane = column parity): ACT
      pair-drains (2 banks/op) into fp16 slabs + one DVE 2x-mode
      tensor_tensor fold per pair for three chunks, DVE direct PSUM
      folds (f32 lanes) for the fourth; engines balanced at ~25-26us
      busy; single-slab granularity at the lead-in and tail to
      shorten the pipeline's serial ends.
    - DMA the folded lane arrays out (~24KB/core), no on-device top-k.
  Host post: top-8 (lane, pos) cells per (row, core) from the folded
  arrays; candidates = {core*8192 + (2k+lane)*512 + pos, k<8} -> 512
  per row; fp32 cosine re-rank, exact fp64 re-rank of the top 16,
  smallest-index tie-break, gather rows.

Screening margin: fp8 dot noise sigma ~9 units (of 4096-scaled sims),
gap between the global max and the 8th-best folded position is ~15
sigma, and the true argmax position is by construction the top-1
folded value of its core, so top-8 position selection cannot lose it
short of astronomically unlikely noise.
"""

import numpy as np
import ml_dtypes

import concourse.bacc as bacc
import concourse.tile as tile
from concourse import mybir
from concourse.bass_utils import run_bass_kernel_spmd

B, D, M_TOT = 512, 512, 65536
C = 8                    # cores
M = M_TOT // C           # 8192 rows per core
P = 128
TB = B // P              # 4 b-chunks
NG = M // 512            # 16 m-groups of 512
QSCALE = 64.0            # pre-quantization scale (exact power of 2)

F32 = mybir.dt.float32
FP16 = mybir.dt.float16
FP8 = mybir.dt.float8e4
U32 = mybir.dt.uint32
DR = mybir.MatmulPerfMode.DoubleRow
MAX = mybir.AluOpType.max

# chunk 3: pairs >= ACT_PAIRS3 are direct DVE drain-folds; everything else
# is ACT pair-drained into fp16 slabs and pair-folded on DVE.
ACT_PAIRS3 = 1
SLAB_OFF = {0: 0, 1: 16, 2: 32, 3: 48}

_NC_CACHE = {}


def build_nc():
    if "nc" in _NC_CACHE:
        return _NC_CACHE["nc"]
    from contextlib import ExitStack

    nc = bacc.Bacc("TRN2", target_bir_lowering=False, debug=False)
    ctx_d = nc.dram_tensor("ctxT8", [P, 4, B], FP8, kind="ExternalInput")
    mem_d = nc.dram_tensor("memT8", [P, 4, M], FP8, kind="ExternalInput")
    rb_d = nc.dram_tensor("rb", [P, 3, 2, 512], FP16, kind="ExternalOutput")
    rb3_d = nc.dram_tensor("rb3", [P, 2, 512], F32, kind="ExternalOutput")

    with tile.TileContext(nc) as tc, ExitStack() as ex:
        big = ex.enter_context(tc.tile_pool(name="big", bufs=1))
        ps = ex.enter_context(tc.tile_pool(name="ps", bufs=1, space="PSUM"))

        ctx8 = big.tile([P, 4, B], FP8)
        mem8 = big.tile([P, 4, M], FP8)
        simb = big.tile([P, 64, 512], FP16)     # ACT-drained slabs
        run2 = big.tile([P, 2, 512], F32)       # chunk-3 direct fold lanes
        runq = big.tile([P, 3, 2, 512], FP16)   # 2-lane running max, c0..c2

        acc = [ps.tile([P, 2, 512], F32, tag=f"acc{b}", name=f"acc{b}")
               for b in range(TB)]

        for b in range(TB):
            bs = slice(b * P, (b + 1) * P)
            nc.sync.dma_start(ctx8[:, :, bs], ctx_d[:, :, bs])
        for k in range(NG):
            nc.gpsimd.dma_start(mem8[:, :, k * 512:(k + 1) * 512],
                                mem_d[:, :, k * 512:(k + 1) * 512])

        def fold_slab(b, s):
            # fold slab s (column g==s) into lane s%2 of the running buffer
            lane = runq[:, b, s % 2, :]
            slab = simb[:, SLAB_OFF[b] + s, :]
            if s <= 1:
                nc.vector.tensor_copy(lane, slab)
            else:
                nc.vector.tensor_tensor(lane, slab, lane, MAX)

        TAIL_ORD = (0, 1, 2, 3)
        for g in range(NG):
            sl = g % 2
            for b in range(TB):
                a = acc[b][:, sl, :]
                ms = slice(g * 512, (g + 1) * 512)
                bs = slice(b * P, (b + 1) * P)
                nc.tensor.matmul(a, ctx8[:, 0:2, bs], mem8[:, 0:2, ms],
                                 start=True, stop=False, perf_mode=DR)
                nc.tensor.matmul(a, ctx8[:, 2:4, bs], mem8[:, 2:4, ms],
                                 start=False, stop=True, perf_mode=DR)
            if g <= 1:
                # lead-in: ACT single drains for c0/c1 (earliest ACT start),
                # DVE drains c2/c3 straight from PSUM in parallel
                for b in range(2):
                    nc.scalar.copy(simb[:, SLAB_OFF[b] + g, :],
                                   acc[b][:, sl, :])
                    fold_slab(b, g)
                nc.vector.tensor_copy(runq[:, 2, g, :], acc[2][:, sl, :])
                nc.vector.tensor_copy(run2[:, g, :], acc[3][:, sl, :])
                continue
            if g >= 14:
                # tail: single-slab drains/folds; c3 folds straight from
                # PSUM into its f32 lane, everyone else via ACT drains
                nc.vector.tensor_tensor(run2[:, sl, :], acc[3][:, sl, :],
                                        run2[:, sl, :], MAX)
                for b in range(3):
                    nc.scalar.copy(simb[:, SLAB_OFF[b] + g, :],
                                   acc[b][:, sl, :])
                    fold_slab(b, g)
                if g == 15:
                    nc.sync.dma_start(rb3_d[:], run2[:])
                    for b in range(3):
                        nc.sync.dma_start(rb_d[:, b, :, :], runq[:, b, :, :])
                continue
            if sl == 1:
                pair = g // 2
                # chunk 3: direct DVE drain-fold for later pairs
                if pair >= ACT_PAIRS3:
                    nc.vector.tensor_tensor(run2[:], acc[3][:],
                                            run2[:], MAX)
                for b in range(TB):
                    if b == 3 and pair >= ACT_PAIRS3:
                        continue
                    s = SLAB_OFF[b] + 2 * pair
                    nc.scalar.copy(simb[:, s:s + 2, :], acc[b][:])
                    # one 2-lane fold per drained pair
                    nc.vector.tensor_tensor(runq[:, b, :, :],
                                            simb[:, s:s + 2, :],
                                            runq[:, b, :, :], MAX)

    nc.compile()
    _NC_CACHE["nc"] = nc
    return nc


def _host_prep(context, memory):
    ctx = np.ascontiguousarray(context, dtype=np.float32)
    mem = np.ascontiguousarray(memory, dtype=np.float32)
    mem_n2 = np.maximum((mem * mem).sum(1, keepdims=True), 1e-12)
    mem_n = mem / np.sqrt(mem_n2)
    ctx_n2 = np.maximum((ctx * ctx).sum(1, keepdims=True), 1e-12)
    ctx_n = ctx / np.sqrt(ctx_n2)

    ctx8 = (ctx_n * QSCALE).astype(ml_dtypes.float8_e4m3)
    mem8 = (mem_n * QSCALE).astype(ml_dtypes.float8_e4m3)

    ctxT8 = np.ascontiguousarray(
        ctx8.T.reshape(4, P, B).transpose(1, 0, 2))
    mem_shards = []
    for c in range(C):
        q = mem8[c * M:(c + 1) * M]
        mem_shards.append(np.ascontiguousarray(
            q.T.reshape(4, P, M).transpose(1, 0, 2)))
    return ctx_n, mem_n, ctxT8, mem_shards


def run_device(context, memory, trace=False):
    nc = build_nc()
    _, _, ctxT8, mem_shards = _host_prep(context, memory)
    in_maps = [{"ctxT8": ctxT8, "memT8": mem_shards[c]} for c in range(C)]
    return run_bass_kernel_spmd(nc, in_maps, list(range(C)), trace=trace)


def kernel(context: np.ndarray, memory: np.ndarray) -> np.ndarray:
    nc = build_nc()
    ctx_n, mem_n, ctxT8, mem_shards = _host_prep(context, memory)
    in_maps = [{"ctxT8": ctxT8, "memT8": mem_shards[c]} for c in range(C)]
    res = run_bass_kernel_spmd(nc, in_maps, list(range(C)))

    # 2-lane folded max arrays; lane = column parity. Host top-8 over the
    # 1024 (lane, pos) cells -> 8 groups of multiplicity each.
    rb = np.stack([res.results[c]["rb"] for c in range(C)], axis=0)
    rb3 = np.stack([res.results[c]["rb3"] for c in range(C)], axis=0)
    rb_b = np.empty((B, C, 2 * 512), dtype=np.float32)
    for tb in range(3):
        rb_b[tb * P:(tb + 1) * P] = (
            rb[:, :, tb, :, :].reshape(C, P, 1024).transpose(1, 0, 2))
    rb_b[3 * P:4 * P] = rb3.reshape(C, P, 1024).transpose(1, 0, 2)
    cell = np.argpartition(-rb_b, 7, axis=2)[:, :, :8].astype(np.int64)
    lane, pos = cell // 512, cell % 512
    k = np.arange(NG // 2, dtype=np.int64)
    cand = (np.arange(C, dtype=np.int64)[None, :, None, None] * M
            + (2 * k[None, None, None, :] + lane[:, :, :, None]) * 512
            + pos[:, :, :, None]).reshape(B, C * 8 * (NG // 2))

    # fp32 cosine prefilter over the 1024 candidates per row
    KTOP = 16
    best16 = np.empty((B, KTOP), dtype=np.int64)
    for b0 in range(0, B, 64):
        b1 = b0 + 64
        rows = mem_n[cand[b0:b1]]                      # [64, K, D] f32
        sc = np.einsum("bd,bkd->bk", ctx_n[b0:b1], rows)
        part = np.argpartition(-sc, KTOP - 1, axis=1)[:, :KTOP]
        best16[b0:b1] = np.take_along_axis(cand[b0:b1], part, axis=1)

    # exact fp64 re-rank of the survivors, smallest-index tie-break
    ctx64 = context.astype(np.float64)
    mem64 = memory.astype(np.float64)
    ctxn64 = ctx64 / np.sqrt(np.maximum((ctx64 * ctx64).sum(1, keepdims=True),
                                        1e-12))
    mnorm = np.sqrt(np.maximum((mem64 * mem64).sum(1), 1e-12))
    rows64 = mem64[best16]                             # [B, 16, D]
    cos = np.einsum("bd,bkd->bk", ctxn64, rows64) / mnorm[best16]
    best = np.empty(B, dtype=np.int64)
    for b in range(B):
        cb, vb = best16[b], cos[b]
        mx = vb.max()
        best[b] = cb[vb >= mx].min()
    return memory[best][None, :, :].astype(np.float32)

